# revision 31
# baseline (speedup 1.0000x reference)
"""GIN-style GNN message-passing layer on 8 Trainium2 NeuronCores.

Math (per reference):
    m      = h[src] + edge_attr                       [E, 96]
    aggr   = segment_sum(m, dst, N)                   [N, 96]
    out    = (1+eps)*h + relu(aggr @ W1 + b1) @ W2 + b2

Distribution strategy (node-parallel, zero collectives):
  Destination nodes are packed on the host into 400 "windows" of <=128 nodes;
  core k owns 50 windows. Every edge belongs to exactly one window (its dst),
  so aggregation is core-local. Per window the device:
    - gathers h[src] rows with the GPSIMD gather-DMA (int16 indices; the
      32767 index limit is handled by splitting each window's edges into
      src<25000 and src>=25000 streams, the second gathered through an
      offset view of the table). Gather calls rotate across the 4 SWDGE
      queues so descriptor generation runs on all four Q7 DSP pairs
      concurrently, and trailing pad indices are -1 so the Q7 ucode trims
      them (descriptor work scales with actual edges, not padded capacity),
    - pre-adds edge_attr into the gathered rows on DVE (messages),
    - builds a [slot, node] one-hot dst indicator on DVE (iota == dst_rel),
    - scatter-adds via TensorE directly in transposed form:
      PSUM[emb, node] += msgs_chunk^T-free matmul (lhsT=msgs, rhs=indicator),
      so the MLP consumes aggr^T with no explicit transpose step.
  The per-node MLP runs on the 128-node window; the residual stream
  (1+eps)*h + b2 is folded on the host. Host un-permutes the shards.

  Windows are dealt to cores by load rank so the j-th window of every core
  has a near-identical chunk count; the per-position static chunk count is
  the max over the 8 cores (SPMD-uniform program, minimal padding).
"""
import os
import numpy as np
import ml_dtypes

import concourse.bass as bass
import concourse.mybir as mybir
import concourse.tile as tile
from concourse import bacc
from concourse.bass_utils import run_bass_kernel_spmd

# problem shape (hardcoded per contest contract)
N_NODES = 50000
N_EDGES = 800000
EMB = 96
HID = 192
P = 128
N_CORES = 8
W_PER_CORE = 50
N_WINDOWS = N_CORES * W_PER_CORE
SPLIT = 25000

N_QUEUES = int(os.environ.get("GNN_QUEUES", "4"))
TRIM = os.environ.get("GNN_TRIM", "1") == "1"

LAST_RESULTS = None      # BassKernelResults of the most recent run (for test.py)
_PROGRAM_CACHE = {}


# ----------------------------------------------------------------- host plan
def _pack_windows(deg_lo, deg_hi, n_windows, cap_half, max_nodes=P):
    order = np.argsort(-(deg_lo + deg_hi), kind="stable")
    lo_left = np.full(n_windows, cap_half, dtype=np.int64)
    hi_left = np.full(n_windows, cap_half, dtype=np.int64)
    slots_left = np.full(n_windows, max_nodes, dtype=np.int64)
    win_of_node = np.full(len(deg_lo), -1, dtype=np.int64)
    ptr = 0
    for v in order:
        dl, dh = deg_lo[v], deg_hi[v]
        for off in range(n_windows):
            w = (ptr + off) % n_windows
            if slots_left[w] > 0 and lo_left[w] >= dl and hi_left[w] >= dh:
                win_of_node[v] = w
                slots_left[w] -= 1
                lo_left[w] -= dl
                hi_left[w] -= dh
                ptr = (w + 1) % n_windows
                break
        else:
            return None
    return win_of_node


def _build_plan(src, dst):
    src = np.asarray(src).astype(np.int64)
    dst = np.asarray(dst).astype(np.int64)
    is_hi = src >= SPLIT

    deg_lo = np.bincount(dst[~is_hi], minlength=N_NODES)
    deg_hi = np.bincount(dst[is_hi], minlength=N_NODES)

    base = max(1, int(np.ceil(max(deg_lo.sum(), deg_hi.sum()) / N_WINDOWS / P)))
    win_of_node = None
    for c in range(base, 40):
        win_of_node = _pack_windows(deg_lo, deg_hi, N_WINDOWS, c * P)
        if win_of_node is not None:
            break
    assert win_of_node is not None, "window packing failed"

    # dense slot of each node inside its window
    order = np.argsort(win_of_node, kind="stable")
    starts = np.searchsorted(win_of_node[order], np.arange(N_WINDOWS))
    slot_sorted = np.arange(N_NODES) - starts[win_of_node[order]]
    slot_of_node = np.empty(N_NODES, dtype=np.int64)
    slot_of_node[order] = slot_sorted

    # per-(window,half) edge counts
    ew0 = win_of_node[dst]
    ekey0 = ew0 * 2 + is_hi
    cnt0 = np.bincount(ekey0, minlength=2 * N_WINDOWS)
    n_lo0, n_hi0 = cnt0[0::2], cnt0[1::2]

    # deal windows to cores by total-load rank: position j on core k is the
    # (8j+k)-th heaviest window, so per-position counts are core-uniform.
    rank = np.argsort(-(n_lo0 + n_hi0), kind="stable")   # heavy first
    # win_remap[old_window] = new_global_id (= core*W_PER_CORE + position)
    win_remap = np.empty(N_WINDOWS, dtype=np.int64)
    for j in range(W_PER_CORE):
        for k in range(N_CORES):
            win_remap[rank[j * N_CORES + k]] = k * W_PER_CORE + j
    win_of_node = win_remap[win_of_node]

    ew = win_of_node[dst]
    ekey = ew * 2 + is_hi
    cnt = np.bincount(ekey, minlength=2 * N_WINDOWS)
    n_lo = cnt[0::2].reshape(N_CORES, W_PER_CORE)
    n_hi = cnt[1::2].reshape(N_CORES, W_PER_CORE)

    # static per-position chunk counts: max over cores at each position
    clo = np.maximum(1, np.ceil(n_lo / P).astype(np.int64).max(axis=0))
    chi = np.maximum(1, np.ceil(n_hi / P).astype(np.int64).max(axis=0))
    nch = clo + chi                       # chunks per position j
    ch_off = np.concatenate([[0], np.cumsum(nch)])  # chunk offset per position
    tot_ch = int(ch_off[-1])              # chunks per core (uniform)

    # slot id of each edge inside its core's slot space:
    #   window position j, lo block at ch_off[j]*P, hi block after clo[j] chunks
    eorder = np.argsort(ekey, kind="stable")
    within = np.arange(N_EDGES) - np.repeat(
        np.concatenate([[0], np.cumsum(cnt)[:-1]]), cnt)
    pos = (ew % W_PER_CORE)
    block_base = np.where(
        is_hi, (ch_off[pos] + clo[pos]) * P, ch_off[pos] * P)
    core_of_edge = ew // W_PER_CORE
    slot_in_core = np.empty(N_EDGES, dtype=np.int64)
    slot_in_core[eorder] = block_base[eorder] + within

    n_slots = tot_ch * P
    # per-core slot tables
    edge_at = np.full((N_CORES, n_slots), -1, dtype=np.int64)
    edge_at[core_of_edge, slot_in_core] = np.arange(N_EDGES)

    # within each (window, half) block, order slots by ascending src so the
    # gather's HBM reads are near-sequential (DRAM row-buffer friendly);
    # pads (src=+inf) stay trailing for the ucode's negative-index trim.
    blk_of_slot = np.zeros(n_slots, dtype=np.int64)
    b = 0
    for j in range(W_PER_CORE):
        lo_end = (ch_off[j] + clo[j]) * P
        hi_end = ch_off[j + 1] * P
        blk_of_slot[ch_off[j] * P:lo_end] = b
        blk_of_slot[lo_end:hi_end] = b + 1
        b += 2
    BIG = np.int64(1) << 32
    for k in range(N_CORES):
        e = edge_at[k]
        subkey = np.where(e < 0, BIG - 1, src[np.where(e < 0, 0, e)])
        order = np.argsort(blk_of_slot * BIG + subkey, kind="stable")
        edge_at[k] = e[order]

    return dict(win_of_node=win_of_node, slot_of_node=slot_of_node,
                clo=clo, chi=chi, nch=nch, ch_off=ch_off, tot_ch=tot_ch,
                edge_at=edge_at)


def _wrap16(idx_flat):
    """[num] -> [128, num//16] int16 (16-partition wrap, x8 replicate)."""
    num = idx_flat.shape[0]
    t = idx_flat.reshape(num // 16, 16).T.astype(np.int16)
    return np.tile(t, (8, 1))


# -------------------------------------------------------------- device build
def _gather96(nc, out_ap, in_ap, idxs_ap, num_idxs, queue_num):
    """dma_gather with elem_size=96 (192B payload, 256B row stride).

    bass.dma_gather asserts elem_size_bytes % 256 == 0, but that restriction
    is only needed for the transpose path / the 8-bit stride encoding (the
    stride here stays 256B = elem_step 128). Emitting the instruction
    directly saves 25% of gather DMA traffic (h rows are 96 wide).
    """
    g = nc.gpsimd
    _in_ap = g.lower_ap_dma(in_ap, for_custom_bir_dma=True)
    _idxs_ap = g.lower_ap(idxs_ap)
    _out_ap = g.lower_ap(out_ap)
    return g.add_instruction(mybir.InstDMAGatherAnt(
        name=g.bass.get_next_instruction_name(),
        ins=[*_in_ap, _idxs_ap, g.lower_val_access(g.to_reg(num_idxs))],
        outs=[_out_ap],
        transpose=False, num_idxs=num_idxs, elem_size=EMB,
        stride_bytes_256=1, gen_mode=0, single_packet=True,
        queue_num=queue_num,
        sbuf_tokens_per_rank=0, sbuf_free_dim_per_rank=0,
        sbuf_free_dim_pad_per_rank=0, sbuf_byte_offset=0))


def _build_program(clo, chi):
    clo = tuple(int(x) for x in clo)
    chi = tuple(int(x) for x in chi)
    nch = [a + b for a, b in zip(clo, chi)]
    ch_off = [0]
    for n in nch:
        ch_off.append(ch_off[-1] + n)
    tot_ch = ch_off[-1]
    cmax = max(nch)
    f32 = mybir.dt.float32
    mdt = mybir.dt.bfloat16

    nc = bacc.Bacc("TRN2", target_bir_lowering=False, debug=False,
                   num_devices=N_CORES, num_swdge_queues=N_QUEUES)
    t_htable = nc.dram_tensor("h_table", [N_NODES, 128], mdt, kind="ExternalInput")
    t_iota = nc.dram_tensor("iota", [P, cmax * P], mdt, kind="ExternalInput")
    t_ea = nc.dram_tensor("ea", [P, tot_ch * EMB], mdt, kind="ExternalInput")
    # per chunk: 8 cols of wrapped gather idx (int16) + 1 col dstrel (bf16 bits)
    t_comb = nc.dram_tensor("comb", [P, tot_ch * 9], mybir.dt.int16,
                            kind="ExternalInput")
    t_w1 = nc.dram_tensor("w1", [EMB, HID], mdt, kind="ExternalInput")
    t_b1 = nc.dram_tensor("b1", [HID, 1], f32, kind="ExternalInput")
    t_w2 = nc.dram_tensor("w2", [HID, EMB], mdt, kind="ExternalInput")
    t_out = nc.dram_tensor("out", [W_PER_CORE * P, EMB], mdt, kind="ExternalOutput")

    with tile.TileContext(nc) as tc:
        with (
            tc.tile_pool(name="const", bufs=1) as cpool,
            tc.tile_pool(name="gath", bufs=12) as gpool,
            tc.tile_pool(name="ea", bufs=12) as epool,
            tc.tile_pool(name="ind", bufs=6) as ipool,
            tc.tile_pool(name="small", bufs=8) as spool,
            tc.tile_pool(name="psuma", bufs=2, space="PSUM") as ppool_a,
            tc.tile_pool(name="psumb", bufs=2, space="PSUM") as ppool_b,
            tc.tile_pool(name="psumc", bufs=2, space="PSUM") as ppool_c,
        ):
            # constants on the Activation HWDGE queue so the sync queue can
            # start streaming window 0's gather indices immediately
            iota_f = cpool.tile([P, cmax * P], mdt)
            nc.scalar.dma_start(out=iota_f[:], in_=t_iota[:])
            w1_t = cpool.tile([EMB, HID], mdt)
            nc.scalar.dma_start(out=w1_t[:], in_=t_w1[:])
            w2a_t = cpool.tile([EMB, EMB], mdt)
            nc.scalar.dma_start(out=w2a_t[:], in_=t_w2[0:EMB, :])
            w2b_t = cpool.tile([EMB, EMB], mdt)
            nc.scalar.dma_start(out=w2b_t[:], in_=t_w2[EMB:HID, :])
            b1a = cpool.tile([EMB, 1], f32)
            nc.scalar.dma_start(out=b1a[:], in_=t_b1[0:EMB, :])
            b1b = cpool.tile([EMB, 1], f32)
            nc.scalar.dma_start(out=b1b[:], in_=t_b1[EMB:HID, :])

            qload = [0] * N_QUEUES
            def pick_q(n_idx):
                q = qload.index(min(qload))
                qload[q] += n_idx
                return q

            for w in range(W_PER_CORE):
                cl, ch = clo[w], chi[w]
                n_w = cl + ch
                off = ch_off[w]

                comb_t = spool.tile([P, cmax * 9], mybir.dt.int16, tag="comb")
                nc.sync.dma_start(out=comb_t[:, 0:n_w * 9],
                                  in_=t_comb[:, off * 9:(off + n_w) * 9])
                gath = gpool.tile([P, cmax, EMB], mdt, tag="gath")
                _gather96(nc, gath[:, 0:cl, :], t_htable[:, 0:EMB],
                          comb_t[:, 0:cl * 8], cl * P, pick_q(cl))
                _gather96(nc, gath[:, cl:n_w, :], t_htable[SPLIT:, 0:EMB],
                          comb_t[:, cl * 8:n_w * 8], ch * P, pick_q(ch))

                # ea in two halves so each pre-add (and the lo-chunk matmuls)
                # can start as soon as its own gather + ea stream lands
                ea_t = epool.tile([P, cmax, EMB], mdt, tag="ea")
                nc.sync.dma_start(
                    out=ea_t[:, 0:cl, :],
                    in_=t_ea[:, off * EMB:(off + cl) * EMB].rearrange(
                        "p (c e) -> p c e", c=cl))
                nc.sync.dma_start(
                    out=ea_t[:, cl:n_w, :],
                    in_=t_ea[:, (off + cl) * EMB:(off + n_w) * EMB].rearrange(
                        "p (c e) -> p c e", c=ch))
                dst_t = comb_t[:, n_w * 8:n_w * 9].bitcast(mdt)

                # messages: m = h[src] + ea (in place in the gather tile)
                nc.vector.tensor_tensor(
                    out=gath[:, 0:cl, :], in0=gath[:, 0:cl, :],
                    in1=ea_t[:, 0:cl, :], op=mybir.AluOpType.add)
                nc.vector.tensor_tensor(
                    out=gath[:, cl:n_w, :], in0=gath[:, cl:n_w, :],
                    in1=ea_t[:, cl:n_w, :], op=mybir.AluOpType.add)

                # dst indicator: [slot, node] one-hot
                ind = ipool.tile([P, cmax, P], mdt, tag="ind")
                nc.vector.tensor_tensor(
                    out=ind[:, 0:n_w, :],
                    in0=dst_t.to_broadcast([P, n_w, P]),
                    in1=iota_f[:].rearrange("p (c j) -> p c j", c=cmax)[:, 0:n_w, :],
                    op=mybir.AluOpType.is_equal)

                # scatter-add via TensorE, transposed: aggrT[emb, node]
                aggrT_p = ppool_a.tile([EMB, P], f32, tag="aggrT")
                for c in range(n_w):
                    nc.tensor.matmul(aggrT_p[:], lhsT=gath[:, c, :],
                                     rhs=ind[:, c, :],
                                     start=(c == 0), stop=(c == n_w - 1))
                aggrT_s = spool.tile([EMB, P], mdt, tag="aggrT_s")
                nc.scalar.copy(aggrT_s[:], aggrT_p[:])

                h1_p = ppool_c.tile([EMB, P], f32, tag="h1")
                nc.tensor.matmul(h1_p[:], lhsT=w1_t[:, 0:EMB], rhs=aggrT_s[:],
                                 start=True, stop=True)
                h2_p = ppool_c.tile([EMB, P], f32, tag="h2")
                nc.tensor.matmul(h2_p[:], lhsT=w1_t[:, EMB:HID], rhs=aggrT_s[:],
                                 start=True, stop=True)
                h1_s = spool.tile([EMB, P], mdt, tag="h1s")
                nc.scalar.activation(h1_s[:], h1_p[:],
                                     mybir.ActivationFunctionType.Relu, bias=b1a[:])
                h2_s = spool.tile([EMB, P], mdt, tag="h2s")
                nc.scalar.activation(h2_s[:], h2_p[:],
                                     mybir.ActivationFunctionType.Relu, bias=b1b[:])

                out_p = ppool_b.tile([P, EMB], f32, tag="outp")
                nc.tensor.matmul(out_p[:], lhsT=h1_s[:], rhs=w2a_t[:],
                                 start=True, stop=False)
                nc.tensor.matmul(out_p[:], lhsT=h2_s[:], rhs=w2b_t[:],
                                 start=False, stop=True)

                # device emits mlp_out only (bf16); the residual
                # (1+eps)*h + b2 is added on the host after unpermute
                out_t = spool.tile([P, EMB], mdt, tag="out")
                nc.scalar.copy(out_t[:], out_p[:])
                nc.scalar.dma_start(out=t_out[w * P:(w + 1) * P, :], in_=out_t[:])

    nc.compile()
    return nc


# ------------------------------------------------------------------- kernel
def kernel(h, edge_attr, src, dst, W1, b1, W2, b2, eps):
    global LAST_RESULTS
    h = np.asarray(h, dtype=np.float32)
    edge_attr = np.asarray(edge_attr, dtype=np.float32)
    W1 = np.asarray(W1, dtype=np.float32)
    b1 = np.asarray(b1, dtype=np.float32)
    W2 = np.asarray(W2, dtype=np.float32)
    b2 = np.asarray(b2, dtype=np.float32)
    eps = np.asarray(eps, dtype=np.float32)
    src_i = np.asarray(src).astype(np.int64)
    dst_i = np.asarray(dst).astype(np.int64)

    plan = _build_plan(src_i, dst_i)
    clo, chi, ch_off, tot_ch = plan["clo"], plan["chi"], plan["ch_off"], plan["tot_ch"]
    key = (tuple(clo), tuple(chi))
    if key not in _PROGRAM_CACHE:
        _PROGRAM_CACHE[key] = _build_program(clo, chi)
    nc = _PROGRAM_CACHE[key]

    n_slots = tot_ch * P
    edge_at = plan["edge_at"]                      # [N_CORES, n_slots]
    pad = edge_at < 0
    e_safe = np.where(pad, 0, edge_at)

    # edge_attr per slot, p-major: [core, P, tot_ch*EMB]
    ea_slots = edge_attr[e_safe].astype(ml_dtypes.bfloat16)
    ea_slots[pad] = 0
    ea_pm = np.ascontiguousarray(
        ea_slots.reshape(N_CORES, tot_ch, P, EMB).transpose(0, 2, 1, 3)
    ).reshape(N_CORES, P, tot_ch * EMB)

    # dst_rel per slot (-1 for pads), p-major [core, P, tot_ch], bf16 bits
    dstrel = np.where(pad, -1.0,
                      plan["slot_of_node"][dst_i[e_safe]]).astype(ml_dtypes.bfloat16)
    dstrel_pm = np.ascontiguousarray(
        dstrel.reshape(N_CORES, tot_ch, P).transpose(0, 2, 1)).view(np.int16)

    # gather indices: hi chunks index the offset table; pads -> -1 (trimmed by
    # ucode) except the first 3 positions per core, whose tile buffers may
    # hold uninitialized SBUF (pads there gather row 0 instead).
    src_slot = src_i[e_safe]
    is_hi_chunk = np.zeros(tot_ch, dtype=bool)
    for j in range(W_PER_CORE):
        is_hi_chunk[ch_off[j] + clo[j]:ch_off[j + 1]] = True
    hi_slot = np.repeat(is_hi_chunk, P)[None, :]
    gidx = np.where(hi_slot, src_slot - SPLIT, src_slot)
    pad_val = -1 if TRIM else 0
    for k in range(N_CORES):
        for j in range(W_PER_CORE):
            pv = 0 if j < 12 else pad_val
            sl = slice(ch_off[j] * P, ch_off[j + 1] * P)
            gidx[k, sl] = np.where(pad[k, sl], pv, gidx[k, sl])
    assert gidx.min() >= -1 and gidx.max() < 32768

    # combined per-window stream: wrapped gather idx (8 cols/chunk) + dstrel
    comb = np.empty((N_CORES, P, tot_ch * 9), dtype=np.int16)
    for j in range(W_PER_CORE):
        a, b = ch_off[j], ch_off[j + 1]
        n_w = b - a
        blk = comb[:, :, a * 9:b * 9]
        for k in range(N_CORES):
            blk[k, :, 0:n_w * 8] = _wrap16(gidx[k, a * P:b * P])
        blk[:, :, n_w * 8:] = dstrel_pm[:, :, a:b]

    h_table = np.zeros((N_NODES, 128), dtype=ml_dtypes.bfloat16)
    h_table[:, :EMB] = h.astype(ml_dtypes.bfloat16)

    cmax = int(np.max(plan["nch"]))
    iota_host = np.tile(np.arange(P, dtype=np.float32), (P, cmax)).astype(
        ml_dtypes.bfloat16)

    in_maps = []
    for k in range(N_CORES):
        in_maps.append(dict(
            h_table=h_table, ea=ea_pm[k], comb=comb[k], iota=iota_host,
            w1=W1.astype(ml_dtypes.bfloat16), b1=b1[:, None],
            w2=W2.astype(ml_dtypes.bfloat16)))

    LAST_RESULTS = run_bass_kernel_spmd(nc, in_maps, core_ids=list(range(N_CORES)),
                                        tmpdir=os.environ.get("GNN_TRACE_DIR") or None)
    shards = np.concatenate([LAST_RESULTS.results[k]["out"]
                             for k in range(N_CORES)], axis=0)
    mlp_out = shards[plan["win_of_node"] * P + plan["slot_of_node"]].astype(np.float32)
    out = (1.0 + eps[0]) * h + b2[None, :] + mlp_out
    return np.ascontiguousarray(out, dtype=np.float32)


MSG_BF16 = True  # kept for test.py's printout
